# revision 2
# baseline (speedup 1.0000x reference)
"""AttentiveFPConv GNN message-passing kernel for 8 Trainium2 NeuronCores.

Reference computation (all fp32):
    alpha = sigmoid(x[col] @ Wa_w + Wa_b)          # per-edge attention
    neigh = x[col] * alpha                          # per-edge message
    aggr  = segment_sum(neigh, row, N)              # per-node aggregation
    out   = tanh(x @ Wn_w + Wn_b + aggr @ Wg_w + Wg_b)

Key algebraic identity: alpha depends only on the source node, so
    h = x * sigmoid(x @ Wa_w + Wa_b)                # per-NODE tensor
    aggr[n] = sum_{e: row[e]=n} h[col[e]]           # gather + segment-sum

Sharding: destination-node sharding. Core k owns nodes [5000k, 5000(k+1))
and ALL edges targeting them. No collective needed.

Per-core pipeline (everything stays transposed: [feature, node]):
  Phase 1: hT = xT * sigmoid(Wa^T xT) for ALL nodes (replicated);
           PE-transpose each 9984-node piece and write h row-major to HBM
           with a PARTITION-MAJOR row permutation so each partition's DMA
           line is 78 rows x 256B contiguous (full-bandwidth writes).
           Host remaps gather indices through the same permutation.
  Phase 2: dma_gather h[col] in destination-sorted edge order (4 SWDGE
           queues; A/B index streams split at permuted row 32768 for the
           int16 index limit). Segment-sum via one-hot matmuls into PSUM
           aggrT [D, 512] per 4-block group. One-hot M tiles are built
           ON DEVICE by a DVE is_equal against an iota row pattern
           (replaces the 23MB/core one-hot stream from HBM).
  Phase 3 (fused per group): poT = Wn^T xT_own + Wg^T aggrT (PSUM
           accumulate), outT = tanh(poT + bias) with the per-feature bias
           applied by the activation unit. outT [D, 5000] f32 to HBM;
           host transposes.
"""

import numpy as np
import ml_dtypes

BF16 = ml_dtypes.bfloat16

# ---------------------------------------------------------------- parameters


class P:
    """Problem/kernel parameters (full-size defaults; shrinkable for tests)."""

    def __init__(self, N=40000, D=128, NCORES=8, HSPLIT=32768,
                 GCHUNK=1024, NQ=4, PIECE=9984, MT=8, SCRATCH=16384):
        assert D == 128
        self.N, self.D, self.NCORES = N, D, NCORES
        self.NB = N // NCORES                 # nodes per core
        self.HSPLIT = HSPLIT                  # h-row split for int16 gather idx
        self.GCHUNK = GCHUNK                  # idxs per dma_gather call
        self.GT = GCHUNK // 128               # gather tiles per chunk
        self.NQ = NQ                          # SWDGE queues for dma_gather
        self.PIECE = PIECE                    # nodes per phase-1 piece
        self.PT = PIECE // 128                # transpose tiles per piece
        self.MT = MT                          # one-hot tiles built per DVE op
        self.NBLK = (self.NB + 127) // 128    # 128-node blocks per core
        self.SCRATCH = SCRATCH                # SWDGE descriptor carveout bytes


def hrow_of_node(p: P, n: np.ndarray) -> np.ndarray:
    """h_d row index for node n under the partition-major piece layout."""
    n = np.asarray(n, np.int64)
    q = n // p.PIECE
    i = n - q * p.PIECE
    t, pp = i // 128, i % 128
    r = q * p.PIECE + pp * p.PT + t
    full = p.N // p.PIECE            # number of full pieces
    return np.where(q < full, r, n)  # tail nodes keep identity rows


# ------------------------------------------------------------ host edge prep


def prep_edges(p: P, row: np.ndarray, col: np.ndarray):
    """Per-core destination-sorted, block-padded edge streams (A/B split by
    permuted h-row < HSPLIT)."""
    row = np.asarray(row).astype(np.int64)
    col = np.asarray(col).astype(np.int64)
    hrow = hrow_of_node(p, col)

    cores = []
    for k in range(p.NCORES):
        sel = (row // p.NB) == k
        r = (row[sel] - k * p.NB).astype(np.int32)
        c = hrow[sel].astype(np.int32)
        order = np.argsort(r, kind="stable")
        r, c = r[order], c[order]
        lo = np.searchsorted(r, np.arange(p.NBLK) * 128)
        hi = np.searchsorted(r, np.minimum(np.arange(1, p.NBLK + 1) * 128, p.NB))
        blocks = []
        for b in range(p.NBLK):
            rb = r[lo[b]:hi[b]] - b * 128
            cb = c[lo[b]:hi[b]]
            mA = cb < p.HSPLIT
            blocks.append(((cb[mA], rb[mA]), (cb[~mA] - p.HSPLIT, rb[~mA])))
        cores.append(blocks)

    nA = np.array([[len(cores[k][b][0][0]) for b in range(p.NBLK)]
                   for k in range(p.NCORES)])
    nB = np.array([[len(cores[k][b][1][0]) for b in range(p.NBLK)]
                   for k in range(p.NCORES)])
    tA = np.maximum(1, -(-nA.max(axis=0) // 128))          # [NBLK]
    tB = np.maximum(1, -(-nB.max(axis=0) // 128))

    LA, LB = int(tA.sum()) * 128, int(tB.sum()) * 128
    LAg = -(-LA // p.GCHUNK) * p.GCHUNK
    LBg = -(-LB // p.GCHUNK) * p.GCHUNK

    per_core = []
    for k in range(p.NCORES):
        idxA = np.zeros(LAg, np.int16); lrA = np.full(LA, -1.0, np.float32)
        idxB = np.zeros(LBg, np.int16); lrB = np.full(LB, -1.0, np.float32)
        oA = oB = 0
        for b in range(p.NBLK):
            (cA, rA), (cB, rB) = cores[k][b]
            idxA[oA:oA + len(cA)] = cA; lrA[oA:oA + len(rA)] = rA
            oA += int(tA[b]) * 128
            idxB[oB:oB + len(cB)] = cB; lrB[oB:oB + len(rB)] = rB
            oB += int(tB[b]) * 128
        per_core.append({
            "idxA": np.tile(idxA.reshape(-1, 16).T, (8, 1)),   # [128, LAg/16]
            "idxB": np.tile(idxB.reshape(-1, 16).T, (8, 1)),
            "lrA": np.ascontiguousarray(
                lrA.reshape(-1, 128).T).astype(BF16),          # [128, LA/128]
            "lrB": np.ascontiguousarray(
                lrB.reshape(-1, 128).T).astype(BF16),
        })
    return tA, tB, LA, LB, LAg, LBg, per_core


# ------------------------------------------------------------- device kernel


def build(p: P, tA, tB, LA, LB, LAg, LBg):
    from concourse import bacc, mybir, tile

    f32, bf16, i16 = mybir.dt.float32, mybir.dt.bfloat16, mybir.dt.int16
    AF = mybir.ActivationFunctionType
    nc = bacc.Bacc("TRN2", target_bir_lowering=False, debug=False,
                   num_devices=p.NCORES, num_swdge_queues=p.NQ,
                   dynamic_dma_scratch_size=p.SCRATCH)

    N, D, NB, NBLK = p.N, p.D, p.NB, p.NBLK
    H = p.HSPLIT
    PIECE, PT, MT = p.PIECE, p.PT, p.MT
    NFULL = N // PIECE                     # full pieces
    TAIL = N - NFULL * PIECE               # tail nodes (plain rows)

    xT_d = nc.dram_tensor("xT", [D, N], bf16, kind="ExternalInput")
    xTo_d = nc.dram_tensor("xT_own", [D, NB], bf16, kind="ExternalInput")
    WaW_d = nc.dram_tensor("WaW", [D, D], bf16, kind="ExternalInput")
    WnW_d = nc.dram_tensor("WnW", [D, D], bf16, kind="ExternalInput")
    WgW_d = nc.dram_tensor("WgW", [D, D], bf16, kind="ExternalInput")
    WaB_d = nc.dram_tensor("WaB", [D, 1], f32, kind="ExternalInput")
    bias_d = nc.dram_tensor("biasT", [D, 1], f32, kind="ExternalInput")
    ident_d = nc.dram_tensor("ident", [D, D], bf16, kind="ExternalInput")
    io_d = nc.dram_tensor("iorep", [128, MT * 128], bf16, kind="ExternalInput")
    idxA_d = nc.dram_tensor("idxA", [128, LAg // 16], i16, kind="ExternalInput")
    idxB_d = nc.dram_tensor("idxB", [128, LBg // 16], i16, kind="ExternalInput")
    lrA_d = nc.dram_tensor("lrA", [128, LA // 128], bf16, kind="ExternalInput")
    lrB_d = nc.dram_tensor("lrB", [128, LB // 128], bf16, kind="ExternalInput")
    outT_d = nc.dram_tensor("outT", [D, NB], f32, kind="ExternalOutput")
    h_d = nc.dram_tensor("h", [N, D], bf16, kind="Internal")

    with tile.TileContext(nc) as tc:
        with (
            tc.tile_pool(name="const", bufs=1) as cpool,
            tc.tile_pool(name="xchunk", bufs=3) as xpool,
            tc.tile_pool(name="hT", bufs=2) as htpool,
            tc.tile_pool(name="hstage", bufs=2) as hspool,
            tc.tile_pool(name="sT", bufs=3) as sTpool,
            tc.tile_pool(name="pg", bufs=2, space="PSUM") as pg_pool,
            tc.tile_pool(name="pt", bufs=1, space="PSUM") as pt_pool,
            tc.tile_pool(name="pa", bufs=2, space="PSUM") as pa_pool,
            tc.tile_pool(name="po", bufs=2, space="PSUM") as po_pool,
            tc.tile_pool(name="sA", bufs=12) as gApool,
            tc.tile_pool(name="sB", bufs=6) as gBpool,
            tc.tile_pool(name="m", bufs=4) as mpool,
            tc.tile_pool(name="agg", bufs=2) as aggpool,
            tc.tile_pool(name="ot", bufs=2) as opool,
            tc.tile_pool(name="tail", bufs=1) as tlpool,
        ):
            # ---- constants into SBUF
            WaW = cpool.tile([D, D], bf16); nc.sync.dma_start(out=WaW[:], in_=WaW_d[:])
            WnW = cpool.tile([D, D], bf16); nc.sync.dma_start(out=WnW[:], in_=WnW_d[:])
            WgW = cpool.tile([D, D], bf16); nc.sync.dma_start(out=WgW[:], in_=WgW_d[:])
            WaB = cpool.tile([D, 1], f32); nc.sync.dma_start(out=WaB[:], in_=WaB_d[:])
            biasT = cpool.tile([D, 1], f32); nc.sync.dma_start(out=biasT[:], in_=bias_d[:])
            ident = cpool.tile([D, D], bf16); nc.sync.dma_start(out=ident[:], in_=ident_d[:])
            iorep = cpool.tile([128, MT * 128], bf16)
            nc.sync.dma_start(out=iorep[:], in_=io_d[:])
            xT_own = cpool.tile([D, NB], bf16); nc.sync.dma_start(out=xT_own[:], in_=xTo_d[:])
            idxA_sb = cpool.tile([128, LAg // 16], i16)
            nc.sync.dma_start(out=idxA_sb[:], in_=idxA_d[:])
            idxB_sb = cpool.tile([128, LBg // 16], i16)
            nc.sync.dma_start(out=idxB_sb[:], in_=idxB_d[:])
            lrA_sb = cpool.tile([128, LA // 128], bf16)
            nc.sync.dma_start(out=lrA_sb[:], in_=lrA_d[:])
            lrB_sb = cpool.tile([128, LB // 128], bf16)
            nc.sync.dma_start(out=lrB_sb[:], in_=lrB_d[:])

            # ---- phase 1: hT = xT * sigmoid(Wa^T xT); transpose; h -> HBM
            for q in range(NFULL + (1 if TAIL else 0)):
                base = q * PIECE
                cn = min(PIECE, N - base)
                hTp = htpool.tile([D, PIECE], bf16, tag="hT")
                off = 0
                while off < cn:
                    w = min(2048, cn - off)
                    xc = xpool.tile([D, 2048], bf16, tag="xc")
                    nc.sync.dma_start(out=xc[:, :w], in_=xT_d[:, base + off:base + off + w])
                    g0 = 0
                    while g0 < w:
                        gw = min(512, w - g0)
                        pg = pg_pool.tile([D, 512], f32, tag="pg")
                        nc.tensor.matmul(pg[:, :gw], lhsT=WaW[:],
                                         rhs=xc[:, g0:g0 + gw], start=True, stop=True)
                        sT = sTpool.tile([D, 512], bf16, tag="sT")
                        nc.scalar.activation(sT[:, :gw], pg[:, :gw], AF.Sigmoid,
                                             bias=WaB[:, 0:1])
                        nc.vector.tensor_tensor(out=hTp[:, off + g0:off + g0 + gw],
                                                in0=xc[:, g0:g0 + gw],
                                                in1=sT[:, :gw], op=mybir.AluOpType.mult)
                        g0 += gw
                    off += w
                if cn == PIECE:
                    # PE-transpose 128-node tiles; stage partition-major
                    hst = hspool.tile([128, PT, 128], bf16, tag="hst")
                    t0 = 0
                    while t0 < PT:
                        tn = min(4, PT - t0)
                        pt = pt_pool.tile([128, 512], bf16, tag="pt")
                        for qq in range(tn):
                            nc.tensor.transpose(
                                pt[:, qq * 128:(qq + 1) * 128],
                                hTp[:, (t0 + qq) * 128:(t0 + qq + 1) * 128], ident[:])
                        nc.scalar.activation(
                            hst[:, t0:t0 + tn, :].rearrange("p t d -> p (t d)"),
                            pt[:, :tn * 128], AF.Copy)
                        t0 += tn
                    # rows base + p*PT + t  <->  hst[p, t, :]  (partition-major)
                    nc.sync.dma_start(
                        out=h_d[base:base + PIECE, :].rearrange(
                            "(p t) d -> p t d", p=128),
                        in_=hst[:, :, :])
                else:
                    # tail: plain rows base+i
                    pt = pt_pool.tile([128, 512], bf16, tag="pt")
                    nc.tensor.transpose(pt[:cn, :128], hTp[:, :cn], ident[:])
                    tl = tlpool.tile([128, 128], bf16, tag="tl")
                    nc.scalar.activation(tl[:cn, :], pt[:cn, :128], AF.Copy)
                    nc.sync.dma_start(out=h_d[base:base + cn, :], in_=tl[:cn, :])

            # ---- phase 2+3: gather, one-hot scatter, node-wise linears
            nq_counter = [0]
            gA_tiles = [None] * (LAg // p.GCHUNK)
            gB_tiles = [None] * (LBg // p.GCHUNK)
            mA_tiles = [None] * (-(-(LA // 128) // MT))
            mB_tiles = [None] * (-(-(LB // 128) // MT))

            def ensure_chunk(which, ci):
                tiles = gA_tiles if which == "A" else gB_tiles
                if tiles[ci] is not None:
                    return
                g = (gApool if which == "A" else gBpool).tile(
                    [128, p.GT, D], bf16, tag="g" + which)
                idx_sb = idxA_sb if which == "A" else idxB_sb
                src = h_d[:H, :] if which == "A" else h_d[H:N, :]
                c0 = ci * (p.GCHUNK // 16)
                nc.gpsimd.dma_gather(
                    out_ap=g[:], in_ap=src, idxs_ap=idx_sb[:, c0:c0 + p.GCHUNK // 16],
                    num_idxs=p.GCHUNK, num_idxs_reg=p.GCHUNK, elem_size=D,
                    queue_num=nq_counter[0] % p.NQ)
                nq_counter[0] += 1
                tiles[ci] = g

            def ensure_mchunk(which, ci):
                tiles = mA_tiles if which == "A" else mB_tiles
                if tiles[ci] is not None:
                    return
                lr_sb = lrA_sb if which == "A" else lrB_sb
                nt = lr_sb.shape[1]
                t0 = ci * MT
                tn = min(MT, nt - t0)
                mt = mpool.tile([128, MT, 128], bf16, tag="m" + which)
                nc.vector.tensor_tensor(
                    out=mt[:, :tn, :].rearrange("p t d -> p (t d)"),
                    in0=iorep[:, :tn * 128],
                    in1=lr_sb[:, t0:t0 + tn].broadcast_to([128, tn, 128]),
                    op=mybir.AluOpType.is_equal)
                tiles[ci] = mt

            posA = np.concatenate([[0], np.cumsum(tA)]).astype(int)
            posB = np.concatenate([[0], np.cumsum(tB)]).astype(int)

            b0 = 0
            while b0 < NBLK:
                gn = min(4, NBLK - b0)
                pa = pa_pool.tile([D, 512], f32, tag="pa")
                for qb in range(gn):
                    b = b0 + qb
                    kA, kB = int(tA[b]), int(tB[b])
                    for j in range(kA + kB):
                        which = "A" if j < kA else "B"
                        g = (posA[b] + j) if j < kA else (posB[b] + j - kA)
                        ensure_chunk(which, g // p.GT)
                        ensure_mchunk(which, g // MT)
                        gt = (gA_tiles if which == "A" else gB_tiles)[g // p.GT]
                        mt = (mA_tiles if which == "A" else mB_tiles)[g // MT]
                        nc.tensor.matmul(pa[:, qb * 128:(qb + 1) * 128],
                                         lhsT=gt[:, g % p.GT, :],
                                         rhs=mt[:, g % MT, :],
                                         start=(j == 0), stop=(j == kA + kB - 1))
                agg = aggpool.tile([D, 512], bf16, tag="agg")
                nc.scalar.activation(agg[:], pa[:], AF.Copy)

                w = min(512, NB - b0 * 128)
                po = po_pool.tile([D, 512], f32, tag="po")
                nc.tensor.matmul(po[:, :w], lhsT=WnW[:],
                                 rhs=xT_own[:, b0 * 128:b0 * 128 + w],
                                 start=True, stop=False)
                nc.tensor.matmul(po[:, :w], lhsT=WgW[:], rhs=agg[:, :w],
                                 start=False, stop=True)
                ot = opool.tile([D, 512], f32, tag="ot")
                nc.scalar.activation(ot[:, :w], po[:, :w], AF.Tanh,
                                     bias=biasT[:, 0:1])
                nc.sync.dma_start(out=outT_d[:, b0 * 128:b0 * 128 + w],
                                  in_=ot[:, :w])
                b0 += gn

    nc.compile()
    return nc


# ---------------------------------------------------------------- host entry


def _host_prep(p: P, x, edge_index, Wn_w, Wn_b, Wg_w, Wg_b, Wa_w, Wa_b):
    x = np.asarray(x, np.float32)
    xT = np.ascontiguousarray(x.T).astype(BF16)
    tA, tB, LA, LB, LAg, LBg, per_core = prep_edges(
        p, np.asarray(edge_index)[0], np.asarray(edge_index)[1])

    iorep = np.tile(np.arange(128, dtype=np.float32), (128, p.MT)).astype(BF16)
    shared = {
        "xT": xT,
        "WaW": np.asarray(Wa_w, np.float32).astype(BF16),
        "WnW": np.asarray(Wn_w, np.float32).astype(BF16),
        "WgW": np.asarray(Wg_w, np.float32).astype(BF16),
        "WaB": np.asarray(Wa_b, np.float32).reshape(p.D, 1),
        "biasT": (np.asarray(Wn_b, np.float32)
                  + np.asarray(Wg_b, np.float32)).reshape(p.D, 1),
        "ident": np.eye(p.D, dtype=np.float32).astype(BF16),
        "iorep": iorep,
    }
    in_maps = []
    for k in range(p.NCORES):
        m = dict(shared)
        m["xT_own"] = np.ascontiguousarray(xT[:, k * p.NB:(k + 1) * p.NB])
        pc = per_core[k]
        m["idxA"], m["idxB"] = pc["idxA"], pc["idxB"]
        m["lrA"], m["lrB"] = pc["lrA"], pc["lrB"]
        in_maps.append(m)
    return tA, tB, LA, LB, LAg, LBg, in_maps


TRACE = False      # set True (e.g. from test.py) to capture an NTFF profile
LAST = None        # last BassKernelResults, for profiling/inspection


def kernel(**inputs) -> np.ndarray:
    global LAST
    from concourse import bass_utils
    bass_utils.upload_artifacts = lambda tmpdir: "local://" + tmpdir

    p = P()
    tA, tB, LA, LB, LAg, LBg, in_maps = _host_prep(p, **inputs)
    nc = build(p, tA, tB, LA, LB, LAg, LBg)
    kw = dict(trace=True, trace_cores=list(range(p.NCORES))) if TRACE else {}
    res = bass_utils.run_bass_kernel_spmd(
        nc, in_maps, core_ids=list(range(p.NCORES)), **kw)
    LAST = res
    out = np.concatenate(
        [res.results[k]["outT"].T for k in range(p.NCORES)], axis=0)
    return np.ascontiguousarray(out).astype(np.float32)
